# revision 50
# baseline (speedup 1.0000x reference)
"""CRF Viterbi decode (torchcrf semantics) on 8 Trainium2 NeuronCores.

Strategy: pure data parallel over batch (1024 rows -> 128 rows/core, one row
per SBUF partition).  Per core:

  Forward (DVE only, scores kept resident in SBUF, bit-exact vs the jax ref):
    cand[b,j,i] = score[b,i] + trans[i,j]          (stride-0 broadcast TT add)
    premax[b,j] = max_i cand[b,j,i]                (segmented tensor_reduce)
    score'[b,j] = premax[b,j] + em[b,t,j]          (small TT add)

  Backward (recomputes each backpointer instead of storing them):
    32x32-block vector-transpose of onehot(tag_{s+1}) -> 4 diagonal
    tile_position matmuls gather transsel[b,i] = trans[i, tag_{s+1}(b)];
    tmp = (score_s + transsel) + em_{s+1}[tag] via one fused ln_bwd_dx
    (associations match the ref exactly); tensor_reduce max; then the NEXT
    step's one-hot comes from match_replace (replace the FIRST argmax cell
    with -BIG -- hardware first-match == jnp.argmax first-index tie break)
    + a tensor_scalar is_equal, which avoids the max_index -> is_equal
    semaphore stall of the naive chain.  max_index still emits the int tag
    for the output but sits off the critical path.

  Device notes (probed on real TRN2 via tiny kernels): the native
  InstTensorTensorReduce is NOT supported by the device lowering, and a PE
  matmul with start=False does NOT accumulate onto DVE-written PSUM on real
  hardware (zero-on-first-touch), so both of those "optimizations" are
  avoided; only custom-DVE TTR / ln_bwd_dx / match_replace /
  tensor_single_scalar are used.

Inputs are taken at full shape; sharding/gather happens on host inside
kernel().
"""

import os
import sys

import numpy as np

if "/opt/trn_rl_repo" not in sys.path:
    sys.path.insert(0, "/opt/trn_rl_repo")

FWD_VARIANT = os.environ.get("CRF_FWD", "base")  # "base" | "split"
BWD_VARIANT = os.environ.get("CRF_BWD", "psum")  # "base" | "mr" | "psum"
TAG_GP = os.environ.get("CRF_TAGGP", "0") == "1"
TRANS_FIRST = os.environ.get("CRF_TF", "0") == "1"

B, T, K = 1024, 1024, 32
NCORES = 8
BL = B // NCORES  # 128 batch rows per core
TC = 128  # time chunk (em streaming / tags8 chunking)
NEG_BIG = -3.0e38
POS_BIG = 3.0e38
JD = 15  # forward j-slice on DVE; [JD, K) computed by GPSIMD (adds only)


def build_nc(t_steps: int = T, tc: int = TC, stage: int = 99):
    """Build + compile the per-core Bass program (same NEFF on all 8 cores)."""
    import concourse.bass as bass
    import concourse.tile as tile
    from concourse import bacc, mybir

    f32 = mybir.dt.float32
    u32 = mybir.dt.uint32
    i32 = mybir.dt.int32
    Alu = mybir.AluOpType
    Ax = mybir.AxisListType

    nsteps = t_steps
    nchunks = (nsteps + tc - 1) // tc
    assert nsteps % tc == 0

    nc = bacc.Bacc(
        "TRN2", target_bir_lowering=False, debug=False, enable_asserts=False
    )

    em_d = nc.dram_tensor("em", [BL, nsteps * K], f32, kind="ExternalInput").ap()
    ttb_d = nc.dram_tensor("ttb", [BL, K * K], f32, kind="ExternalInput").ap()
    tmov_d = nc.dram_tensor("tmov", [128, K], f32, kind="ExternalInput").ap()
    endt_d = nc.dram_tensor("endt", [BL, K], f32, kind="ExternalInput").ap()
    iota_d = nc.dram_tensor("iota", [BL, K], u32, kind="ExternalInput").ap()
    tags_d = nc.dram_tensor("tags", [BL, nsteps], i32, kind="ExternalOutput").ap()

    with tile.TileContext(nc) as tc_ctx:
        _body(nc, tc_ctx, bass, mybir, Alu, Ax, f32, u32, i32,
              em_d, ttb_d, tmov_d, endt_d, iota_d, tags_d, nsteps, tc, nchunks,
              stage)

    nc.compile()
    return nc


def _body(nc, tc_ctx, bass, mybir, Alu, Ax, f32, u32, i32,
          em_d, ttb_d, tmov_d, endt_d, iota_d, tags_d, nsteps, tc, nchunks,
          stage=99):
    from contextlib import ExitStack

    ctx = ExitStack()
    with ctx:
        const_pool = ctx.enter_context(tc_ctx.tile_pool(name="const", bufs=1))
        hist_pool = ctx.enter_context(tc_ctx.tile_pool(name="hist", bufs=1))
        em_pool = ctx.enter_context(tc_ctx.tile_pool(name="em", bufs=2))
        work_pool = ctx.enter_context(tc_ctx.tile_pool(name="work", bufs=1))
        tags8_pool = ctx.enter_context(tc_ctx.tile_pool(name="tags8", bufs=2))
        mrb_pool = ctx.enter_context(tc_ctx.tile_pool(name="mrb", bufs=1))
        psum_pool = ctx.enter_context(
            tc_ctx.tile_pool(
                name="psum",
                bufs=1 if BWD_VARIANT == "psum" else 2,
                space="PSUM",
            )
        )

        # ---- constants ----
        ttb = const_pool.tile([BL, K * K], f32)  # ttb[b, j*K+i] = trans[i, j]
        nc.sync.dma_start(ttb[:], ttb_d[:])
        tmov = const_pool.tile([128, K], f32)  # trans.T tiled x4 over partitions
        nc.sync.dma_start(tmov[:], tmov_d[:])
        endt = const_pool.tile([BL, K], f32)
        nc.sync.dma_start(endt[:], endt_d[:])
        iota = const_pool.tile([BL, K], u32)
        nc.sync.dma_start(iota[:], iota_d[:])
        # fp32 iota + threshold for the bulk tag extraction ("mr" backward)
        iota_f = const_pool.tile([BL, K], f32)
        nc.vector.tensor_copy(iota_f[:], iota[:])
        neg_thr = const_pool.tile([BL, 1], f32)
        nc.vector.memset(neg_thr[:], -1.0e38)

        # ---- working tiles ----
        hist = hist_pool.tile([BL, nsteps * K], f32)  # all forward scores
        cand = work_pool.tile([BL, K * K], f32)
        cand_g = work_pool.tile([BL, (K - JD) * K], f32)
        premax = work_pool.tile([BL, K], f32)
        premax_g = work_pool.tile([BL, K - JD], f32)
        m8 = work_pool.tile([BL, 8], f32)
        u_t = work_pool.tile([BL, K], f32)
        tmp = work_pool.tile([BL, K], f32)
        tagf = work_pool.tile([BL, TC], f32)
        scr = work_pool.tile([BL, K], f32)
        emsel = work_pool.tile([BL, 1], f32)
        onehot = work_pool.tile([BL, K], f32)
        vt = work_pool.tile([BL, K], f32)
        vtr = work_pool.tile([BL, K], f32)
        scr_g = work_pool.tile([BL, K], f32)
        tagout = work_pool.tile([BL, nsteps], i32)

        nc.vector.memset(m8[:], POS_BIG)

        ttb3 = ttb[:].rearrange("p (j i) -> p j i", i=K)
        jd = JD
        jg = K - JD
        # DVE computes cand slice j in [0, jd); GPSIMD computes [jd, K) into
        # its OWN tile (a shared tile creates a false cross-engine hazard).
        # DVE then does both segmented max-reduces (GPSIMD cannot reduce
        # along the free axis) and the em add.
        cand3 = cand[:].rearrange("p (j i) -> p j i", i=K)
        cand_d3 = cand3[:, 0:jd, :]
        cand_g3 = cand_g[:].rearrange("p (j i) -> p j i", i=K)
        ttb_d3 = ttb3[:, 0:jd, :]
        ttb_g3 = ttb3[:, jd:K, :]

        # ================= forward =================
        for c in range(nchunks):
            emf = em_pool.tile([BL, tc * K], f32, tag="emchunk")
            nc.sync.dma_start(emf[:], em_d[:, c * tc * K : (c + 1) * tc * K])
            for tloc in range(tc):
                t = c * tc + tloc
                em_sl = emf[:, tloc * K : (tloc + 1) * K]
                h_t = hist[:, t * K : (t + 1) * K]
                if t == 0:
                    nc.vector.tensor_copy(h_t, em_sl)
                    continue
                h_prev = hist[:, (t - 1) * K : t * K]
                h_prev_b = h_prev[:, None, :]
                if FWD_VARIANT == "base":
                    nc.vector.tensor_tensor(
                        cand3,
                        h_prev_b.broadcast_to([BL, K, K]),
                        ttb3,
                        Alu.add,
                    )
                    nc.vector.tensor_reduce(premax[:], cand3, Ax.X, Alu.max)
                    nc.vector.tensor_tensor(h_t, premax[:], em_sl, Alu.add)
                    continue
                nc.vector.tensor_tensor(
                    cand_d3,
                    h_prev_b.broadcast_to([BL, jd, K]),
                    ttb_d3,
                    Alu.add,
                )
                nc.gpsimd.tensor_tensor(
                    cand_g3,
                    h_prev_b.broadcast_to([BL, jg, K]),
                    ttb_g3,
                    Alu.add,
                )
                with tc_ctx.high_priority():
                    nc.vector.tensor_reduce(
                        premax[:, 0:jd], cand_d3, Ax.X, Alu.max
                    )
                    nc.vector.tensor_tensor(
                        h_t[:, 0:jd], premax[:, 0:jd], em_sl[:, 0:jd], Alu.add
                    )
                nc.vector.tensor_reduce(premax_g[:], cand_g3, Ax.X, Alu.max)
                nc.vector.tensor_tensor(
                    h_t[:, jd:K], premax_g[:], em_sl[:, jd:K], Alu.add
                )

        # ================= final argmax =================
        # ref: score = hist[T-1] + end_transitions, then argmax (first index)
        from concourse.dve_ops import TENSOR_TENSOR_REDUCE as _CTTR

        mr_mode = BWD_VARIANT in ("mr", "psum")
        ps_mode = False  # PE accumulate onto DVE-written PSUM zeroes on HW
        last_slot = (nsteps - 1) - (nchunks - 1) * tc
        tmpr = work_pool.tile([BL, K], f32)
        nc.vector.tensor_tensor(
            tmp[:], hist[:, (nsteps - 1) * K : nsteps * K], endt[:], Alu.add
        )
        tags8_cur = tags8_pool.tile([BL, tc * 8], u32, tag="t8")
        if mr_mode:
            nc.vector.tensor_reduce(m8[:, 0:1], tmp[:], Ax.X, Alu.max)
            nc.vector.match_replace(tmpr[:], m8[:], tmp[:], NEG_BIG)
            if TRANS_FIRST and not TAG_GP:
                nc.vector.transpose(vtr[:], tmpr[:])
                nc.vector.tensor_single_scalar(
                    vt[:], vtr[:], NEG_BIG, Alu.is_equal
                )
            else:
                nc.vector.tensor_single_scalar(
                    onehot[:], tmpr[:], NEG_BIG, Alu.is_equal
                )
        else:
            nc.vector.max(m8[:], tmp[:])
        if TAG_GP and mr_mode:
            nc.gpsimd.scalar_tensor_tensor(
                scr_g[:], onehot[:], 1.0, iota_f[:],
                Alu.mult, Alu.mult,
                accum_out=tagf[:, last_slot : last_slot + 1],
            )
        else:
            nc.vector.max_index(
                tags8_cur[:, last_slot * 8 : last_slot * 8 + 8], m8[:], tmp[:]
            )

        # ================= backward =================
        from concourse.dve_ops import TENSOR_TENSOR_REDUCE as _CTTR

        tags8_by_chunk = {nchunks - 1: tags8_cur}
        bwd_chunks = range(nchunks - 1, -1, -1) if stage >= 3 else [nchunks - 1]
        for c in bwd_chunks:
            # em[s+1] for s in [c*tc, (c+1)*tc): dram slice offset by one step
            n_em = tc if c < nchunks - 1 else tc - 1
            embw = em_pool.tile([BL, tc * K], f32, tag="emchunk")
            nc.sync.dma_start(
                embw[:, : n_em * K],
                em_d[:, (c * tc + 1) * K : (c * tc + 1 + n_em) * K],
            )
            if c not in tags8_by_chunk:
                tags8_by_chunk[c] = tags8_pool.tile(
                    [BL, tc * 8], u32, tag="t8", name=f"t8c{c}"
                )
            t8c = tags8_by_chunk[c]
            s_hi = min(nsteps - 2, (c + 1) * tc - 1)
            for s in (range(s_hi, c * tc - 1, -1) if stage >= 3 else
                      range(s_hi, s_hi - 1, -1)):
                tloc = s - c * tc
                if not mr_mode:
                    # one-hot of tag_{s+1} from the stored tag index
                    sp1 = s + 1
                    cp1 = sp1 // tc
                    t8p = tags8_by_chunk[cp1]
                    slot = sp1 - cp1 * tc
                    nc.vector.tensor_tensor(
                        onehot[:],
                        iota[:],
                        t8p[:, slot * 8 : slot * 8 + 1].broadcast_to([BL, K]),
                        Alu.is_equal,
                    )
                # (for mr/psum modes, onehot of tag_{s+1} is already there)
                if stage >= 4 and not (mr_mode and TRANS_FIRST):
                    nc.vector.transpose(vt[:], onehot[:])
                # transsel[b,i] = trans[i, tag_b] via 4 diagonal 32x32 matmuls
                if ps_mode:
                    tsel = pshist[:, tloc * K : (tloc + 1) * K]
                else:
                    tsel_t = psum_pool.tile([BL, K], f32, tag="tsel")
                    tsel = tsel_t[:]
                if stage >= 5:
                    for r in range(4):
                        nc.tensor.matmul(
                            tsel[32 * r : 32 * r + 32, :],
                            vt[32 * r : 32 * r + 32, :],
                            tmov[32 * r : 32 * r + 32, :],
                            start=not ps_mode,
                            stop=True,
                            tile_position=(32 * r, 32 * r),
                            skip_group_check=ps_mode,
                        )
                else:
                    nc.vector.memset(tsel[:], 0.0)
                em_sl_bw = embw[:, tloc * K : (tloc + 1) * K]
                if BWD_VARIANT == "base":
                    # emselneg[b] = -em_{s+1}[b, tag_{s+1}(b)] (s1=-1)
                    nc.vector._custom_dve(
                        _CTTR,
                        out=scr[:],
                        in0=onehot[:],
                        in1=em_sl_bw,
                        s0=0.0,
                        s1=-1.0,
                        accum_out=emsel[:],
                    )
                    # tmp = (hist_s - tsel*(-1) - (-emsel))*1
                    nc.vector.ln_bwd_dx(
                        tmp[:], hist[:, s * K : (s + 1) * K], tsel[:], -1.0,
                        emsel[:], 1.0,
                    )
                    nc.vector.tensor_reduce(m8[:, 0:1], tmp[:], Ax.X, Alu.max)
                    nc.vector.max_index(
                        t8c[:, tloc * 8 : tloc * 8 + 8], m8[:], tmp[:]
                    )
                    continue
                # emselneg[b] = -em_{s+1}[b, tag_{s+1}(b)]
                nc.vector._custom_dve(
                    _CTTR,
                    out=scr[:],
                    in0=onehot[:],
                    in1=em_sl_bw,
                    s0=0.0,
                    s1=-1.0,
                    accum_out=emsel[:],
                )
                # tmp = (hist_s - tsel*(-1) - (-emsel))*1 = (hist_s+tsel)+emsel
                # (native tensor_tensor_reduce is not supported by the device
                # lowering, so use the fused ln_bwd_dx + a plain max reduce)
                nc.vector.ln_bwd_dx(
                    tmp[:], hist[:, s * K : (s + 1) * K], tsel, -1.0,
                    emsel[:], 1.0,
                )
                nc.vector.tensor_reduce(m8[:, 0:1], tmp[:], Ax.X, Alu.max)
                # next one-hot straight from the max value: replace the first
                # argmax cell with NEG_BIG, then compare (first-match = the
                # ref's first-index argmax tie break)
                nc.vector.match_replace(tmpr[:], m8[:], tmp[:], NEG_BIG)
                if TAG_GP:
                    # tag = sum(onehot * iota) on idle GPSIMD, fully off the
                    # DVE critical path (consumed only at chunk end)
                    nc.vector.tensor_single_scalar(
                        onehot[:], tmpr[:], NEG_BIG, Alu.is_equal
                    )
                    nc.gpsimd.scalar_tensor_tensor(
                        scr_g[:], onehot[:], 1.0, iota_f[:],
                        Alu.mult, Alu.mult,
                        accum_out=tagf[:, tloc : tloc + 1],
                    )
                elif TRANS_FIRST:
                    # transpose the raw mr output, then is_equal on the
                    # transposed values; also is_equal on the raw output for
                    # emsel's one-hot
                    nc.vector.transpose(vtr[:], tmpr[:])
                    nc.vector.tensor_single_scalar(
                        vt[:], vtr[:], NEG_BIG, Alu.is_equal
                    )
                    nc.vector.tensor_single_scalar(
                        onehot[:], tmpr[:], NEG_BIG, Alu.is_equal
                    )
                    nc.vector.max_index(
                        t8c[:, tloc * 8 : tloc * 8 + 8], m8[:], tmp[:]
                    )
                else:
                    nc.vector.tensor_single_scalar(
                        onehot[:], tmpr[:], NEG_BIG, Alu.is_equal
                    )
                    nc.vector.max_index(
                        t8c[:, tloc * 8 : tloc * 8 + 8], m8[:], tmp[:]
                    )

            if TAG_GP and mr_mode:
                # tagf holds exact small ints (f32); cast on ScalarE
                nc.scalar.copy(tagout[:, c * tc : (c + 1) * tc], tagf[:])
            else:
                # compact this chunk's tags (slot stride 8 -> dense)
                t83 = t8c[:].rearrange("p (s e) -> p s e", e=8)
                nc.scalar.copy(
                    tagout[:, c * tc : (c + 1) * tc][:, :, None],
                    t83[:, :, 0:1],
                )
            nc.sync.dma_start(
                tags_d[:, c * tc : (c + 1) * tc], tagout[:, c * tc : (c + 1) * tc]
            )
            if c + 1 in tags8_by_chunk:
                del tags8_by_chunk[c + 1]


_NC_CACHE = {}


def _get_nc(t_steps=T, tc=TC):
    key = (t_steps, tc)
    if key not in _NC_CACHE:
        _NC_CACHE[key] = build_nc(t_steps, tc)
    return _NC_CACHE[key]


def make_in_maps(inputs, start_transitions, end_transitions, transitions,
                 t_steps=T):
    """Host-side shard + constant prep. Returns list of per-core input dicts."""
    inputs = np.asarray(inputs, np.float32)
    start = np.asarray(start_transitions, np.float32)
    end = np.asarray(end_transitions, np.float32)
    trans = np.asarray(transitions, np.float32)

    ttb = np.ascontiguousarray(
        np.broadcast_to(trans.T.reshape(1, K * K), (BL, K * K))
    )
    tmov = np.ascontiguousarray(np.tile(trans.T, (4, 1)))
    endt = np.ascontiguousarray(np.broadcast_to(end.reshape(1, K), (BL, K)))
    iota = np.ascontiguousarray(
        np.broadcast_to(np.arange(K, dtype=np.uint32), (BL, K))
    )

    in_maps = []
    for ci in range(NCORES):
        em = np.array(
            inputs[ci * BL : (ci + 1) * BL, :t_steps].reshape(BL, t_steps * K)
        )
        # fold start_transitions into em[0] (same association as the ref)
        em[:, :K] = start.reshape(1, K) + em[:, :K]
        in_maps.append(
            {"em": em, "ttb": ttb, "tmov": tmov, "endt": endt, "iota": iota}
        )
    return in_maps


_last_result = None


def kernel(inputs, mask, start_transitions, end_transitions, transitions):
    global _last_result
    mask = np.asarray(mask)
    if not mask.all():
        return _numpy_fallback(
            np.asarray(inputs, np.float32), mask,
            np.asarray(start_transitions, np.float32),
            np.asarray(end_transitions, np.float32),
            np.asarray(transitions, np.float32),
        )

    from concourse.bass_utils import run_bass_kernel_spmd

    nc = _get_nc()
    in_maps = make_in_maps(inputs, start_transitions, end_transitions, transitions)
    res = run_bass_kernel_spmd(nc, in_maps, core_ids=list(range(NCORES)))
    _last_result = res
    tags = np.concatenate([res.results[i]["tags"] for i in range(NCORES)], axis=0)
    return tags.astype(np.int32)


def _numpy_fallback(inputs, mask, start, end, trans):
    """Vectorized numpy Viterbi matching torchcrf/ref semantics (general mask)."""
    em = np.swapaxes(inputs, 0, 1)  # [T, B, K]
    mk = np.swapaxes(mask, 0, 1)  # [T, B]
    nT, nB, nK = em.shape
    score = start[None, :] + em[0]
    hist = np.zeros((nT - 1, nB, nK), np.int32)
    for t in range(1, nT):
        cand = score[:, :, None] + trans[None, :, :] + em[t][:, None, :]
        bp = np.argmax(cand, axis=1).astype(np.int32)
        ns = np.max(cand, axis=1)
        m = mk[t][:, None]
        score = np.where(m, ns, score)
        hist[t - 1] = bp
    score = score + end[None, :]
    tag = np.argmax(score, axis=1).astype(np.int32)
    tags = np.zeros((nT, nB), np.int32)
    tags[nT - 1] = tag
    for t in range(nT - 2, -1, -1):
        prev = np.take_along_axis(hist[t], tag[:, None], axis=1)[:, 0]
        prev = np.where(mk[t + 1], prev, tag)
        tags[t] = prev
        tag = prev
    return np.swapaxes(tags, 0, 1).astype(np.int32)

